# revision 1
# baseline (speedup 1.0000x reference)
"""Per-neuron grouped MLP (conv-style) kernel for Trainium2, 8 NeuronCores.

Math (per group d):  h = x[:, d, :] @ W1[d].T + b1[d]; g = gelu(h); out[:, d] = g @ W2[d] + b2[d]
  x: [B=512, D=2048, M=128], W1: [D, H=128, M], b1: [D, H], W2: [D, H], b2: [D]

Strategy:
  - Shard on D: each of 8 cores owns D_LOC = 256 independent per-neuron MLPs.
  - Host pre-transposes so every DMA is contiguous:
      xT[d, m, b], W1T[d, m, h], W2T[h, d], b1T[h, d]
  - Inputs cast to fp16 on host (PSUM accumulation stays fp32): 4x PE stream
    rate vs fp32 and half the DMA bytes, with ~1e-3 worst-case output error.
  - Per quad of 4 d's on-chip (contraction dims on partitions):
      psum1[H, 2B] = W1T[d].T @ xT[d]          (N=512 matmuls, 2 per psum tile)
      g[H, 4B]     = gelu(psum1 + b1[d])       (ScalarE, exact-erf Gelu)
      psum2[128,B] = 4 packed matmuls W2T[:,d].T @ g_d, tile_position=(0,32j)
                     -> rows {0,32,64,96}
      o_sb         = DVE full-tile copy of psum2 (only 4 rows carry data)
      outT[d:d+4] <- strided-partition DMA of o_sb rows {0,32,64,96}
  - b2 added on host (it is outside the nonlinearity).
"""

import numpy as np

B, D, M, H = 512, 2048, 128, 128
N_CORES = 8
D_LOC = D // N_CORES  # 256
QUAD = 4     # d's per MM2 packing group
PAIR = 2     # d's per psum1/ACT batch
SUPER = 16   # d's per super-group: one x DMA, one w1 DMA, one out DMA
# Within a super-group [D0, D0+16), quad c (c=0..3) handles d = D0 + 4j + c
# (j=0..3); MM2 j lands on psum row 32j, so out rows {D0..D0+15} are exactly
# o_sb[0::32, c, :] in (row, quad, b) iteration order -> single strided DMA.

PRECISION = "fp16"  # "fp16" | "fp32"

_NC_CACHE = {}


def build_nc(bias_mode: bool, prec: str = PRECISION, reps: int = 1):
    """Build + compile the Bass module (shared SPMD program for all 8 cores).

    reps>1 replicates the whole body (same in/out DRAM) for benchmarking:
    one NEFF execution then runs the kernel `reps` times back-to-back."""
    key = (bias_mode, prec, reps)
    if key in _NC_CACHE:
        return _NC_CACHE[key]

    import concourse.bacc as bacc
    import concourse.mybir as mybir
    import concourse.tile as tile

    f32 = mybir.dt.float32
    dt = f32 if prec == "fp32" else mybir.dt.float16
    GELU = mybir.ActivationFunctionType.Gelu

    nc = bacc.Bacc("TRN2", target_bir_lowering=False, debug=False, num_devices=N_CORES)
    xT = nc.dram_tensor("xT", [D_LOC, M, B], dt, kind="ExternalInput").ap()
    w1T = nc.dram_tensor("w1T", [D_LOC, M, H], dt, kind="ExternalInput").ap()
    w2T = nc.dram_tensor("w2T", [H, D_LOC], dt, kind="ExternalInput").ap()
    b1T = nc.dram_tensor("b1T", [H, D_LOC], f32, kind="ExternalInput").ap()
    outT = nc.dram_tensor("outT", [D_LOC, B], f32, kind="ExternalOutput").ap()

    with (
        tile.TileContext(nc) as tc,
        tc.tile_pool(name="singles", bufs=1) as singles,
        tc.tile_pool(name="xp", bufs=3) as xp,
        tc.tile_pool(name="wp", bufs=2) as wp,
        tc.tile_pool(name="gp", bufs=4) as gp,
        tc.tile_pool(name="op", bufs=4) as op_pool,
        tc.tile_pool(name="ps1", bufs=2, space="PSUM") as ps1,
        tc.tile_pool(name="ps2", bufs=2, space="PSUM") as ps2,
    ):
        w2_sb = singles.tile([H, D_LOC], dt)
        nc.sync.dma_start(out=w2_sb[:], in_=w2T[:])
        b1_sb = None
        if bias_mode:
            b1_sb = singles.tile([H, D_LOC], f32)
            nc.sync.dma_start(out=b1_sb[:], in_=b1T[:])

        for _rep in range(reps):
            _body_loop(nc, tc, bias_mode, dt, f32, GELU,
                       xT, w1T, outT, w2_sb, b1_sb,
                       xp, wp, gp, op_pool, ps1, ps2)

    nc.compile()
    _NC_CACHE[key] = nc
    return nc


def _body_loop(nc, tc, bias_mode, dt, f32, GELU, xT, w1T, outT, w2_sb, b1_sb,
               xp, wp, gp, op_pool, ps1, ps2):
        for sg in range(D_LOC // SUPER):
            D0 = sg * SUPER
            x_sb = xp.tile([M, SUPER, B], dt)
            nc.sync.dma_start(
                out=x_sb[:],
                in_=xT[D0 : D0 + SUPER].rearrange("d m b -> m d b"),
            )
            w1_sb = wp.tile([M, SUPER, H], dt)
            nc.scalar.dma_start(
                out=w1_sb[:],
                in_=w1T[D0 : D0 + SUPER].rearrange("d m h -> m d h"),
            )
            o_sb = op_pool.tile([128, SUPER // QUAD, B], f32)
            for c in range(SUPER // QUAD):
                # quad c handles d = D0 + (SUPER//QUAD)*j + c, j = 0..3
                g_sb = gp.tile([H, QUAD * B], dt)
                for pr in range(QUAD // PAIR):
                    p1 = ps1.tile([H, PAIR * B], f32)
                    for j in range(PAIR):
                        jj = pr * PAIR + j
                        nc.tensor.matmul(
                            p1[:, j * B : (j + 1) * B],
                            lhsT=w1_sb[:, (SUPER // QUAD) * jj + c, :],
                            rhs=x_sb[:, (SUPER // QUAD) * jj + c, :],
                            start=True,
                            stop=True,
                        )
                    if bias_mode:
                        for j in range(PAIR):
                            jj = pr * PAIR + j
                            dd = D0 + (SUPER // QUAD) * jj + c
                            nc.scalar.activation(
                                g_sb[:, jj * B : (jj + 1) * B],
                                p1[:, j * B : (j + 1) * B],
                                GELU,
                                bias=b1_sb[:, dd : dd + 1],
                            )
                    else:
                        nc.scalar.activation(
                            g_sb[:, pr * PAIR * B : (pr + 1) * PAIR * B], p1[:], GELU
                        )
                p2 = ps2.tile([128, B], f32)
                for j in range(QUAD):
                    dd = D0 + (SUPER // QUAD) * j + c
                    nc.tensor.matmul(
                        p2[32 * j : 32 * j + 1, :],
                        lhsT=w2_sb[:, dd : dd + 1],
                        rhs=g_sb[:, j * B : (j + 1) * B],
                        start=True,
                        stop=True,
                        tile_position=(0, 32 * j),
                    )
                nc.vector.tensor_copy(o_sb[:, c, :], p2[:])
            nc.gpsimd.dma_start(
                out=outT[D0 : D0 + SUPER, :], in_=o_sb[0::32, :, :]
            )


def prepare_in_maps(x, W1, b1, W2, prec: str = PRECISION):
    """Host-side shard + transpose. Returns list of 8 per-core input dicts."""
    np_dt = np.float32 if prec == "fp32" else np.float16
    x = np.asarray(x, dtype=np.float32)
    W1 = np.asarray(W1, dtype=np.float32)
    b1 = np.asarray(b1, dtype=np.float32)
    W2 = np.asarray(W2, dtype=np.float32)

    in_maps = []
    for k in range(N_CORES):
        sl = slice(k * D_LOC, (k + 1) * D_LOC)
        # [B, D_LOC, M] -> [D_LOC, M, B]; see kernel docstring for why.
        xT_k = np.ascontiguousarray(x[:, sl, :].transpose(1, 2, 0), dtype=np_dt)
        w1T_k = np.ascontiguousarray(W1[sl].transpose(0, 2, 1), dtype=np_dt)
        w2T_k = np.ascontiguousarray(W2[sl].T, dtype=np_dt)
        b1T_k = np.ascontiguousarray(b1[sl].T, dtype=np.float32)
        in_maps.append({"xT": xT_k, "w1T": w1T_k, "w2T": w2T_k, "b1T": b1T_k})
    return in_maps


def assemble_output(results, b2):
    outT_full = np.concatenate([r["outT"] for r in results], axis=0)  # [D, B]
    out = outT_full.T  # [B, D]
    b2 = np.asarray(b2, dtype=np.float32)
    if np.any(b2):
        out = out + b2[None, :]
    return np.ascontiguousarray(out)


def kernel(pre_activation_history, W1, b1, W2, b2):
    from concourse.bass_utils import run_bass_kernel_spmd

    b1 = np.asarray(b1, dtype=np.float32)
    bias_mode = bool(np.any(b1))
    nc = build_nc(bias_mode)
    in_maps = prepare_in_maps(pre_activation_history, W1, b1, W2)
    res = run_bass_kernel_spmd(nc, in_maps, core_ids=list(range(N_CORES)))
    return assemble_output(res.results, b2)



# revision 5
# speedup vs baseline: 2.2058x; 2.2058x over previous
"""Per-neuron grouped MLP (conv-style) kernel for Trainium2, 8 NeuronCores.

Math (per group d):  h = x[:, d, :] @ W1[d].T + b1[d]; g = gelu(h); out[:, d] = g @ W2[d] + b2[d]
  x: [B=512, D=2048, M=128], W1: [D, H=128, M], b1: [D, H], W2: [D, H], b2: [D]

Strategy (v2):
  - Shard on D: each of 8 cores owns D_LOC = 256 independent per-neuron MLPs.
  - Host pre-packs x so each chunk DMA is one dense, fully contiguous
    [128, CHUNK*B] transfer (32 KB contiguous per partition line).
  - W1 is fully SBUF-resident ([m, d, h] = 64 KB/partition), loaded in 4
    dense DMAs on the second HWDGE queue so it streams alongside x.
  - Per d: MM1 psum1[H, B] = W1[d].T @ x[d]  (K=M=128, N=512).
    Gelu per PAIR of d's (one ScalarE activation, FD=1024, PSUM source).
    MM2 packs 4 d's into one PSUM bank via col-tiled matmuls
    (tile_position=(0,32j)), then one DVE copy per quad -> SBUF, and one
    strided DMA per chunk writes outT[d, b] fp32.
  - PRECISION:
      "fp16": x cast to fp16 on host.
      "int8": x quantized per-(d,m) column to int8 in DRAM (half the HBM
        traffic); the SWDGE DMA casts int8->fp16 in flight (values +-127 are
        exact in fp16). The dequant scales are folded into W1 on the host
        (h = sum_m x_int * (s_dm * W1[d,h,m])), plus a global 2^7 boost to
        keep folded weights in fp16 normal range, undone by the activation's
        free pre-scale (gelu(psum * 2^-7)).
  - b1 is zero in this problem; a bias-mode fallback applies it per-d.
    b2 is added on the host (it is outside the nonlinearity).
"""

import numpy as np

B, D, M, H = 512, 2048, 128, 128
N_CORES = 8
D_LOC = D // N_CORES  # 256
CHUNK = 32           # d's per x DMA chunk
NCHUNK = D_LOC // CHUNK
QUAD = 4             # d's per MM2 packing group
PAIR = 2             # d's per psum1/ACT group
W1_SLICES = 4        # W1 preload DMA count

PRECISION = "fp16"   # "fp16" | "int8"
INT8_BOOST = 128.0   # power-of-2 fold boost for int8 mode

_NC_CACHE = {}


def build_nc(bias_mode: bool, prec: str = PRECISION, reps: int = 1):
    key = (bias_mode, prec, reps)
    if key in _NC_CACHE:
        return _NC_CACHE[key]

    import concourse.bacc as bacc
    import concourse.mybir as mybir
    import concourse.tile as tile

    f32 = mybir.dt.float32
    dt = mybir.dt.float16
    x_dram_dt = mybir.dt.int8 if prec == "int8" else dt

    nc = bacc.Bacc("TRN2", target_bir_lowering=False, debug=False, num_devices=N_CORES)
    xT = nc.dram_tensor("xT", [NCHUNK, M, CHUNK, B], x_dram_dt, kind="ExternalInput").ap()
    w1T = nc.dram_tensor("w1T", [M, D_LOC, H], dt, kind="ExternalInput").ap()
    w2T = nc.dram_tensor("w2T", [H, D_LOC], dt, kind="ExternalInput").ap()
    b1T = nc.dram_tensor("b1T", [H, D_LOC], f32, kind="ExternalInput").ap()
    outT = nc.dram_tensor("outT", [D_LOC, B], f32, kind="ExternalOutput").ap()

    with (
        tile.TileContext(nc) as tc,
        tc.tile_pool(name="singles", bufs=1) as singles,
        tc.tile_pool(name="xp", bufs=2) as xp,
        tc.tile_pool(name="gp", bufs=3) as gp,
        tc.tile_pool(name="op", bufs=2) as op_pool,
        tc.tile_pool(name="ps1", bufs=3, space="PSUM") as ps1,
        tc.tile_pool(name="ps2", bufs=2, space="PSUM") as ps2,
    ):
        w2_sb = singles.tile([H, D_LOC], dt)
        nc.sync.dma_start(out=w2_sb[:], in_=w2T[:])
        b1_sb = None
        if bias_mode:
            b1_sb = singles.tile([H, D_LOC], f32)
            nc.sync.dma_start(out=b1_sb[:], in_=b1T[:])
        # W1 resident: [m, d, h]. All input DMAs share the SP HWDGE FIFO so
        # issue order = completion order; W1 slices are interleaved between
        # early x chunks by _body (slice s gates chunks 2s..2s+1).
        w1_sb = singles.tile([M, D_LOC, H], dt)

        for _rep in range(reps):
            _body(nc, tc, bias_mode, prec, dt, f32,
                  xT, w1T, outT, w1_sb, w2_sb, b1_sb, xp, gp, op_pool, ps1, ps2)

    nc.compile()
    _NC_CACHE[key] = nc
    return nc


def _body(nc, tc, bias_mode, prec, dt, f32, xT, w1T, outT, w1_sb, w2_sb, b1_sb,
          xp, gp, op_pool, ps1, ps2):
    import concourse.mybir as mybir

    GELU = mybir.ActivationFunctionType.Gelu
    act_scale = (1.0 / INT8_BOOST) if prec == "int8" else 1.0
    NQ = CHUNK // QUAD
    DS = D_LOC // W1_SLICES  # d's per W1 slice (gates chunks c: c*CHUNK < (s+1)*DS)

    def x_dma(out, in_):
        if prec == "int8":
            nc.gpsimd.dma_start(out=out, in_=in_)  # SWDGE casts i8->f16
        else:
            nc.sync.dma_start(out=out, in_=in_)

    def w1_dma(s):
        nc.sync.dma_start(
            out=w1_sb[:, s * DS : (s + 1) * DS, :],
            in_=w1T[:, s * DS : (s + 1) * DS, :],
        )

    for c in range(NCHUNK):
        if c == 0:
            w1_dma(0)  # first W1 slice before anything else
        x_sb = xp.tile([M, CHUNK, B], dt)
        if c == 0:
            # split chunk 0 so compute starts as soon as 8 d's have landed
            for lo, hi in ((0, 8), (8, 16), (16, CHUNK)):
                x_dma(x_sb[:, lo:hi, :], xT[c][:, lo:hi, :])
        else:
            x_dma(x_sb[:], xT[c])
        if 1 <= c < W1_SLICES:
            w1_dma(c)  # interleave remaining W1 slices between x chunks
        o_sb = op_pool.tile([128, NQ, B], f32)
        for q in range(NQ):
            g_sb = gp.tile([H, QUAD * B], dt)
            for pr in range(QUAD // PAIR):
                p1 = ps1.tile([H, PAIR * B], f32)
                for j in range(PAIR):
                    dd = q * QUAD + pr * PAIR + j
                    nc.tensor.matmul(
                        p1[:, j * B : (j + 1) * B],
                        lhsT=w1_sb[:, c * CHUNK + dd, :],
                        rhs=x_sb[:, dd, :],
                        start=True,
                        stop=True,
                    )
                if bias_mode:
                    for j in range(PAIR):
                        dd = q * QUAD + pr * PAIR + j
                        nc.scalar.activation(
                            g_sb[:, (pr * PAIR + j) * B : (pr * PAIR + j + 1) * B],
                            p1[:, j * B : (j + 1) * B],
                            GELU,
                            bias=b1_sb[:, c * CHUNK + dd : c * CHUNK + dd + 1],
                            scale=act_scale,
                        )
                else:
                    nc.scalar.activation(
                        g_sb[:, pr * PAIR * B : (pr + 1) * PAIR * B],
                        p1[:],
                        GELU,
                        scale=act_scale,
                    )
            p2 = ps2.tile([128, B], f32)
            for j in range(QUAD):
                dd = c * CHUNK + q * QUAD + j
                nc.tensor.matmul(
                    p2[32 * j : 32 * j + 1, :],
                    lhsT=w2_sb[:, dd : dd + 1],
                    rhs=g_sb[:, j * B : (j + 1) * B],
                    start=True,
                    stop=True,
                    tile_position=(0, 32 * j),
                )
            nc.vector.tensor_copy(o_sb[:, q, :], p2[:])
        # rows {0,32,64,96} x NQ quads hold out for d = 4*q + j
        nc.gpsimd.dma_start(
            out=outT[c * CHUNK : (c + 1) * CHUNK].rearrange(
                "(q j) b -> j q b", j=QUAD
            ),
            in_=o_sb[0::32, :, :],
        )


def prepare_in_maps(x, W1, b1, W2, prec: str = PRECISION):
    """Host-side shard + pack. Returns list of 8 per-core input dicts."""
    x = np.asarray(x, dtype=np.float32)
    W1 = np.asarray(W1, dtype=np.float32)
    b1 = np.asarray(b1, dtype=np.float32)
    W2 = np.asarray(W2, dtype=np.float32)

    in_maps = []
    for k in range(N_CORES):
        sl = slice(k * D_LOC, (k + 1) * D_LOC)
        xk = x[:, sl, :]          # [B, D_LOC, M]
        w1k = W1[sl]              # [D_LOC, H, M]
        if prec == "int8":
            # per-(d,m) symmetric int8 over b; scales folded into W1
            s = np.max(np.abs(xk), axis=0) / 127.0  # [D_LOC, M]
            s = np.maximum(s, 1e-12)
            xq = np.clip(np.round(xk / s[None]), -127, 127).astype(np.int8)
            xT_k = np.ascontiguousarray(
                xq.transpose(2, 1, 0)  # [M, D_LOC, B]
                .reshape(M, NCHUNK, CHUNK, B)
                .transpose(1, 0, 2, 3)  # [NCHUNK, M, CHUNK, B]
            )
            w1f = w1k * (s[:, None, :] * INT8_BOOST)  # fold scales into W1
        else:
            xT_k = np.ascontiguousarray(
                xk.transpose(2, 1, 0)
                .reshape(M, NCHUNK, CHUNK, B)
                .transpose(1, 0, 2, 3),
                dtype=np.float16,
            )
            w1f = w1k
        # w1T layout [m, d, h]
        w1T_k = np.ascontiguousarray(w1f.transpose(2, 0, 1), dtype=np.float16)
        w2T_k = np.ascontiguousarray(W2[sl].T, dtype=np.float16)
        b1T_k = np.ascontiguousarray(b1[sl].T, dtype=np.float32)
        in_maps.append({"xT": xT_k, "w1T": w1T_k, "w2T": w2T_k, "b1T": b1T_k})
    return in_maps


def assemble_output(results, b2):
    outT_full = np.concatenate([r["outT"] for r in results], axis=0)  # [D, B]
    out = outT_full.T  # [B, D]
    b2 = np.asarray(b2, dtype=np.float32)
    if np.any(b2):
        out = out + b2[None, :]
    return np.ascontiguousarray(out)


def kernel(pre_activation_history, W1, b1, W2, b2):
    from concourse.bass_utils import run_bass_kernel_spmd

    b1 = np.asarray(b1, dtype=np.float32)
    bias_mode = bool(np.any(b1))
    nc = build_nc(bias_mode)
    in_maps = prepare_in_maps(pre_activation_history, W1, b1, W2)
    res = run_bass_kernel_spmd(nc, in_maps, core_ids=list(range(N_CORES)))
    return assemble_output(res.results, b2)
